# revision 4
# baseline (speedup 1.0000x reference)
"""MultiHeadAttention (B=2, S=2048, D=1024, H=16, DK=64, causal) on 8 TRN2 cores.

Sharding: batch x head-group. Core c handles batch b = c//4 and heads
h in [4g, 4g+4) where g = c%4 (data + head/tensor parallel). W_out is
column-sharded per head group; the 4 partial output projections per batch
are summed on the host during unshard (+ bias).

Per-core kernel layout strategy (all on-device transposes avoided):
 - Host feeds x.T ([D, S]) in bf16, so projections contract over D with
   D on SBUF partitions.
 - Q.T, K.T computed as [64, S] per head, two heads packed per PE matmul
   (full 128-wide stationary operand).
 - V computed in natural [S, 64] layout with a ones-column appended, so
   the P.T@V matmul also produces the softmax denominators for free.
 - Scores are computed transposed ([k, q]); softmax runs along the free
   dim after exp; no 2M-element probs transpose is ever needed.
 - Causal handled by skipping fully-masked k-tiles and an additive -1e30
   mask on the 4 diagonal phases.
 - out.T [64, q] per head is exactly the stationary operand the output
   projection needs; normalization by 1/rowsum is applied via a tiny
   ones-broadcast matmul + element-wise multiply.
"""

import numpy as np
import ml_dtypes

import concourse.bass as bass
import concourse.mybir as mybir
from concourse import bacc, tile
from concourse.bass_utils import run_bass_kernel_spmd

B, S, D = 2, 2048, 1024
H, DK = 16, 64
P = 128
HPC = 4          # heads per core
NPAIR = 2        # head pairs per core
DC = D // P      # 8 contraction chunks for projections
KT = S // P      # 16 key tiles
QC = S // 512    # 4 query chunks of 512
NEG = -1.0e30
SCALE = 1.0 / 8.0  # 1/sqrt(DK)

F32 = mybir.dt.float32
BF16 = mybir.dt.bfloat16
BF = ml_dtypes.bfloat16

_CACHE = {}


def _build_program():
    nc = bacc.Bacc("TRN2", target_bir_lowering=False)

    xqT = nc.dram_tensor("xqT", [D, S], BF16, kind="ExternalInput")
    xkT = nc.dram_tensor("xkT", [D, S], BF16, kind="ExternalInput")
    xvT = nc.dram_tensor("xvT", [D, S], BF16, kind="ExternalInput")
    wq = nc.dram_tensor("wq", [D, HPC * DK], BF16, kind="ExternalInput")
    wk = nc.dram_tensor("wk", [D, HPC * DK], BF16, kind="ExternalInput")
    wv = nc.dram_tensor("wv", [D, HPC * DK], BF16, kind="ExternalInput")
    woT = nc.dram_tensor("woT", [HPC * DK, D], BF16, kind="ExternalInput")
    maskT = nc.dram_tensor("maskT", [4, P, 512], F32, kind="ExternalInput")
    y = nc.dram_tensor("y", [S, D], F32, kind="ExternalOutput")

    with tile.TileContext(nc) as tc:
        with (
            tc.tile_pool(name="consts", bufs=1) as consts,
            tc.tile_pool(name="xres", bufs=1) as xres,
            tc.tile_pool(name="qkv", bufs=1) as qkv,
            tc.tile_pool(name="probs", bufs=4) as probs_pool,
            tc.tile_pool(name="small", bufs=3) as small,
            tc.tile_pool(name="yout", bufs=3) as yout,
            tc.tile_pool(name="psum", bufs=8, space="PSUM") as psum_pool,
        ):
            # ---------------- weights / masks / constants ----------------
            # wq/wk/wv staged as [128, DC*256]: chunk c at cols [c*256, c*256+256)
            wq_sb = consts.tile([P, DC * 256], BF16, name="wq_sb")
            wk_sb = consts.tile([P, DC * 256], BF16, name="wk_sb")
            wv_sb = consts.tile([P, DC * 256], BF16, name="wv_sb")
            for c in range(DC):
                nc.sync.dma_start(wq_sb[:, c * 256:(c + 1) * 256], wq[c * P:(c + 1) * P, :])
                nc.sync.dma_start(wk_sb[:, c * 256:(c + 1) * 256], wk[c * P:(c + 1) * P, :])
                nc.sync.dma_start(wv_sb[:, c * 256:(c + 1) * 256], wv[c * P:(c + 1) * P, :])
            woT_sb = []
            for pi in range(NPAIR):
                t = consts.tile([P, D], BF16, name=f"woT_sb{pi}")
                nc.sync.dma_start(t[:], woT[pi * P:(pi + 1) * P, :])
                woT_sb.append(t)
            mask_sb = consts.tile([P, 4 * 512], F32, name="mask_sb")
            for m in range(4):
                nc.sync.dma_start(mask_sb[:, m * 512:(m + 1) * 512], maskT[m, :, :])
            ones64 = consts.tile([1, 64], F32, name="ones64")
            nc.vector.memset(ones64[:], 1.0)

            # ---------------- resident transposed inputs ----------------
            xq_sb, xk_sb, xv_sb = [], [], []
            for c in range(DC):
                tq = xres.tile([P, S], BF16, name=f"xq{c}")
                nc.sync.dma_start(tq[:], xqT[c * P:(c + 1) * P, :])
                xq_sb.append(tq)
            for c in range(DC):
                tk = xres.tile([P, S], BF16, name=f"xk{c}")
                nc.sync.dma_start(tk[:], xkT[c * P:(c + 1) * P, :])
                xk_sb.append(tk)
            for c in range(DC):
                tv = xres.tile([P, S], BF16, name=f"xv{c}")
                nc.sync.dma_start(tv[:], xvT[c * P:(c + 1) * P, :])
                xv_sb.append(tv)

            # ---------------- Q.T / K.T projections (head pairs) ----------------
            qt_sb, kt_sb = [], []
            for pi in range(NPAIR):
                qt_sb.append(qkv.tile([P, S], BF16, name=f"qt{pi}"))
                kt_sb.append(qkv.tile([P, S], BF16, name=f"kt{pi}"))
            for src, w_sb, dst in ((xq_sb, wq_sb, qt_sb), (xk_sb, wk_sb, kt_sb)):
                for pi in range(NPAIR):
                    for n in range(QC):
                        ps = psum_pool.tile([P, 512], F32, tag="ps", name=f"psqk{pi}{n}")
                        for c in range(DC):
                            nc.tensor.matmul(
                                ps[:],
                                w_sb[:, c * 256 + pi * P: c * 256 + (pi + 1) * P],
                                src[c][:, n * 512:(n + 1) * 512],
                                start=(c == 0),
                                stop=(c == DC - 1),
                            )
                        nc.vector.tensor_copy(dst[pi][:, n * 512:(n + 1) * 512], ps[:])

            # ---------------- V projection into [128, KT*(DK+1)] with ones col ----------------
            vones = []
            for h in range(HPC):
                t = qkv.tile([P, KT * (DK + 1)], BF16, name=f"vones{h}")
                nc.vector.memset(t[:], 1.0)
                vones.append(t)
            for i in range(KT):
                ps = psum_pool.tile([P, 256], F32, tag="ps", name=f"psv{i}")
                for c in range(DC):
                    nc.tensor.matmul(
                        ps[:],
                        xv_sb[c][:, i * P:(i + 1) * P],
                        wv_sb[:, c * 256:(c + 1) * 256],
                        start=(c == 0),
                        stop=(c == DC - 1),
                    )
                for h in range(HPC):
                    nc.vector.tensor_copy(
                        vones[h][:, i * (DK + 1): i * (DK + 1) + DK],
                        ps[:, h * DK:(h + 1) * DK],
                    )

            # ---------------- attention + output projection ----------------
            otn_sb = []
            for pi in range(NPAIR):
                otn_sb.append(qkv.tile([P, S], BF16, name=f"otn{pi}"))

            for qc in range(QC):
                qlo = qc * 512
                n_k = 4 * qc + 4
                for h in range(HPC):
                    pi, hp = h // 2, h % 2
                    hs = slice(hp * DK, (hp + 1) * DK)
                    oT = psum_pool.tile([P, 512], F32, tag="ps", name=f"oT{qc}{h}")
                    for i in range(n_k):
                        sc = psum_pool.tile([P, 512], F32, tag="ps", name=f"sc{qc}{h}{i}")
                        nc.tensor.matmul(
                            sc[:],
                            kt_sb[pi][hs, i * P:(i + 1) * P],
                            qt_sb[pi][hs, qlo:qlo + 512],
                            start=True,
                            stop=True,
                        )
                        if i >= 4 * qc:  # diagonal tile: additive causal mask
                            m = i - 4 * qc
                            nc.vector.tensor_add(sc[:], sc[:], mask_sb[:, m * 512:(m + 1) * 512])
                        pr = probs_pool.tile([P, 512], BF16, tag="pr", name=f"pr{qc}{h}{i}")
                        nc.scalar.activation(
                            pr[:], sc[:], mybir.ActivationFunctionType.Exp, scale=SCALE
                        )
                        nc.tensor.matmul(
                            oT[0:DK + 1, :],
                            vones[h][:, i * (DK + 1):(i + 1) * (DK + 1)],
                            pr[:],
                            start=(i == 0),
                            stop=(i == n_k - 1),
                        )
                    # normalize: rows 0..63 are V.T@P.T, row 64 is the row-sums
                    rs = small.tile([1, 512], F32, tag="rs", name=f"rs{qc}{h}")
                    nc.vector.reciprocal(rs[:], oT[DK:DK + 1, :])
                    bc = psum_pool.tile([P, 512], F32, tag="ps", name=f"bc{qc}{h}")
                    nc.tensor.matmul(bc[0:DK, :], ones64[:], rs[:], start=True, stop=True)
                    bcs = small.tile([DK, 512], F32, tag="bcs", name=f"bcs{qc}{h}")
                    nc.scalar.activation(bcs[:], bc[0:DK, :], mybir.ActivationFunctionType.Copy)
                    nc.vector.tensor_mul(
                        otn_sb[pi][hp * DK:(hp + 1) * DK, qlo:qlo + 512],
                        oT[0:DK, :],
                        bcs[:],
                    )
                # output projection for the 4 q-tiles of this chunk
                for t in range(4):
                    qs = qlo + t * P
                    for dchunk in range(2):
                        yp = psum_pool.tile([P, 512], F32, tag="ps", name=f"yp{qc}{t}{dchunk}")
                        for pi in range(NPAIR):
                            nc.tensor.matmul(
                                yp[:],
                                otn_sb[pi][:, qs:qs + P],
                                woT_sb[pi][:, dchunk * 512:(dchunk + 1) * 512],
                                start=(pi == 0),
                                stop=(pi == NPAIR - 1),
                            )
                        ys = yout.tile([P, 512], F32, tag="ys", name=f"ys{qc}{t}{dchunk}")
                        nc.vector.tensor_copy(ys[:], yp[:])
                        nc.sync.dma_start(y[qs:qs + P, dchunk * 512:(dchunk + 1) * 512], ys[:])

    nc.compile()
    return nc


def _mask_phases():
    c = np.arange(512)[None, :]
    p = np.arange(P)[:, None]
    return np.stack(
        [np.where(c >= p + P * m, 0.0, NEG) for m in range(4)]
    ).astype(np.float32)


def _prep_in_maps(query, key, value, Wq, Wk, Wv, Wo):
    xT = {}
    for b in range(B):
        xT[b] = tuple(
            np.ascontiguousarray(np.asarray(a[b], np.float32).T).astype(BF)
            for a in (query, key, value)
        )
    maskT = _mask_phases()
    in_maps = []
    for core in range(8):
        b, g = core // 4, core % 4
        hsl = slice(HPC * g, HPC * (g + 1))
        in_maps.append({
            "xqT": xT[b][0],
            "xkT": xT[b][1],
            "xvT": xT[b][2],
            "wq": np.concatenate(list(np.asarray(Wq, np.float32)[hsl]), axis=1).astype(BF),
            "wk": np.concatenate(list(np.asarray(Wk, np.float32)[hsl]), axis=1).astype(BF),
            "wv": np.concatenate(list(np.asarray(Wv, np.float32)[hsl]), axis=1).astype(BF),
            "woT": np.ascontiguousarray(
                np.asarray(Wo, np.float32)[:, 256 * g:256 * (g + 1)].T
            ).astype(BF),
            "maskT": maskT,
        })
    return in_maps


def run(query, key, value, Wq, Wk, Wv, Wo, bo, trace=False, **trace_kwargs):
    if "nc" not in _CACHE:
        _CACHE["nc"] = _build_program()
    nc = _CACHE["nc"]
    in_maps = _prep_in_maps(query, key, value, Wq, Wk, Wv, Wo)
    res = run_bass_kernel_spmd(nc, in_maps, list(range(8)), trace=trace, **trace_kwargs)
    out = np.zeros((B, S, D), np.float32)
    for core in range(8):
        out[core // 4] += np.asarray(res.results[core]["y"], np.float32)
    out += np.asarray(bo, np.float32)[None, None, :]
    return out, res


def kernel(query, key, value, Wq, Wk, Wv, Wo, bo):
    out, _ = run(query, key, value, Wq, Wk, Wv, Wo, bo)
    return out


# revision 5
# speedup vs baseline: 1.1615x; 1.1615x over previous
"""MultiHeadAttention (B=2, S=2048, D=1024, H=16, DK=64, causal) on 8 TRN2 cores.

Sharding: batch x head-group. Core c handles batch b = c//4 and heads
h in [4g, 4g+4) where g = c%4 (data + head/tensor parallel). W_out is
column-sharded per head group; the 4 partial output projections per batch
are summed on the host during unshard (+ bias).

Per-core kernel layout strategy (all on-device transposes avoided):
 - Host feeds x.T ([D, S]) in bf16, so projections contract over D with
   D on SBUF partitions.
 - Q.T, K.T computed as [64, S] per head, two heads packed per PE matmul
   (full 128-wide stationary operand).
 - V computed in natural [S, 64] layout with a ones-column appended, so
   the P.T@V matmul also produces the softmax denominators for free.
 - Scores are computed transposed ([k, q]); softmax runs along the free
   dim after exp; no 2M-element probs transpose is ever needed.
 - Causal handled by skipping fully-masked k-tiles and an additive -1e30
   mask on the 4 diagonal phases.
 - out.T [64, q] per head is exactly the stationary operand the output
   projection needs; normalization by 1/rowsum is applied via a tiny
   ones-broadcast matmul + element-wise multiply.
"""

import numpy as np
import ml_dtypes

import concourse.bass as bass
import concourse.mybir as mybir
from concourse import bacc, tile
from concourse.bass_utils import run_bass_kernel_spmd

B, S, D = 2, 2048, 1024
H, DK = 16, 64
P = 128
HPC = 4          # heads per core
NPAIR = 2        # head pairs per core
DC = D // P      # 8 contraction chunks for projections
KT = S // P      # 16 key tiles
QC = S // 512    # 4 query chunks of 512
NEG = -1.0e30
SCALE = 1.0 / 8.0  # 1/sqrt(DK)

F32 = mybir.dt.float32
BF16 = mybir.dt.bfloat16
BF = ml_dtypes.bfloat16

_CACHE = {}


def _build_program():
    nc = bacc.Bacc("TRN2", target_bir_lowering=False)

    xqT = nc.dram_tensor("xqT", [D, S], BF16, kind="ExternalInput")
    xkT = nc.dram_tensor("xkT", [D, S], BF16, kind="ExternalInput")
    xvT = nc.dram_tensor("xvT", [D, S], BF16, kind="ExternalInput")
    wq = nc.dram_tensor("wq", [D, HPC * DK], BF16, kind="ExternalInput")
    wk = nc.dram_tensor("wk", [D, HPC * DK], BF16, kind="ExternalInput")
    wv = nc.dram_tensor("wv", [D, HPC * DK], BF16, kind="ExternalInput")
    woT = nc.dram_tensor("woT", [HPC * DK, D], BF16, kind="ExternalInput")
    maskT = nc.dram_tensor("maskT", [4, P, 512], F32, kind="ExternalInput")
    y = nc.dram_tensor("y", [S, D], F32, kind="ExternalOutput")

    with tile.TileContext(nc) as tc:
        with (
            tc.tile_pool(name="consts", bufs=1) as consts,
            tc.tile_pool(name="xres", bufs=1) as xres,
            tc.tile_pool(name="qkv", bufs=1) as qkv,
            tc.tile_pool(name="probs", bufs=4) as probs_pool,
            tc.tile_pool(name="small", bufs=3) as small,
            tc.tile_pool(name="yout", bufs=3) as yout,
            tc.tile_pool(name="psum", bufs=8, space="PSUM") as psum_pool,
        ):
            # ---------------- weights / masks / constants ----------------
            # wq/wk/wv staged as [128, DC*256]: chunk c at cols [c*256, c*256+256)
            wq_sb = consts.tile([P, DC * 256], BF16, name="wq_sb")
            wk_sb = consts.tile([P, DC * 256], BF16, name="wk_sb")
            wv_sb = consts.tile([P, DC * 256], BF16, name="wv_sb")
            for c in range(DC):
                nc.sync.dma_start(wq_sb[:, c * 256:(c + 1) * 256], wq[c * P:(c + 1) * P, :])
                nc.sync.dma_start(wk_sb[:, c * 256:(c + 1) * 256], wk[c * P:(c + 1) * P, :])
                nc.sync.dma_start(wv_sb[:, c * 256:(c + 1) * 256], wv[c * P:(c + 1) * P, :])
            woT_sb = []
            for pi in range(NPAIR):
                t = consts.tile([P, D], BF16, name=f"woT_sb{pi}")
                nc.sync.dma_start(t[:], woT[pi * P:(pi + 1) * P, :])
                woT_sb.append(t)
            mask_sb = consts.tile([P, 4 * 512], F32, name="mask_sb")
            for m in range(4):
                nc.sync.dma_start(mask_sb[:, m * 512:(m + 1) * 512], maskT[m, :, :])
            ones64 = consts.tile([1, 64], F32, name="ones64")
            nc.vector.memset(ones64[:], 1.0)

            # ---------------- resident transposed inputs ----------------
            xq_sb, xk_sb, xv_sb = [], [], []
            for c in range(DC):
                tq = xres.tile([P, S], BF16, name=f"xq{c}")
                nc.sync.dma_start(tq[:], xqT[c * P:(c + 1) * P, :])
                xq_sb.append(tq)
            for c in range(DC):
                tk = xres.tile([P, S], BF16, name=f"xk{c}")
                nc.sync.dma_start(tk[:], xkT[c * P:(c + 1) * P, :])
                xk_sb.append(tk)
            for c in range(DC):
                tv = xres.tile([P, S], BF16, name=f"xv{c}")
                nc.sync.dma_start(tv[:], xvT[c * P:(c + 1) * P, :])
                xv_sb.append(tv)

            # ---------------- Q.T / K.T projections (head pairs) ----------------
            qt_sb, kt_sb = [], []
            for pi in range(NPAIR):
                qt_sb.append(qkv.tile([P, S], BF16, name=f"qt{pi}"))
                kt_sb.append(qkv.tile([P, S], BF16, name=f"kt{pi}"))
            for src, w_sb, dst in ((xq_sb, wq_sb, qt_sb), (xk_sb, wk_sb, kt_sb)):
                for pi in range(NPAIR):
                    for n in range(QC):
                        ps = psum_pool.tile([P, 512], F32, tag="ps", name=f"psqk{pi}{n}")
                        for c in range(DC):
                            nc.tensor.matmul(
                                ps[:],
                                w_sb[:, c * 256 + pi * P: c * 256 + (pi + 1) * P],
                                src[c][:, n * 512:(n + 1) * 512],
                                start=(c == 0),
                                stop=(c == DC - 1),
                            )
                        nc.vector.tensor_copy(dst[pi][:, n * 512:(n + 1) * 512], ps[:])

            # ---------------- V projection into [128, KT*(DK+1)] with ones col ----------------
            vones = []
            for h in range(HPC):
                t = qkv.tile([P, KT * (DK + 1)], BF16, name=f"vones{h}")
                nc.vector.memset(t[:], 1.0)
                vones.append(t)
            for i in range(KT):
                ps = psum_pool.tile([P, 256], F32, tag="ps", name=f"psv{i}")
                for c in range(DC):
                    nc.tensor.matmul(
                        ps[:],
                        xv_sb[c][:, i * P:(i + 1) * P],
                        wv_sb[:, c * 256:(c + 1) * 256],
                        start=(c == 0),
                        stop=(c == DC - 1),
                    )
                for h in range(HPC):
                    nc.vector.tensor_copy(
                        vones[h][:, i * (DK + 1): i * (DK + 1) + DK],
                        ps[:, h * DK:(h + 1) * DK],
                    )

            # ---------------- attention + output projection ----------------
            otn_sb = []
            for pi in range(NPAIR):
                otn_sb.append(qkv.tile([P, S], BF16, name=f"otn{pi}"))

            for qc in range(QC):
                qlo = qc * 512
                n_k = 4 * qc + 4
                for h in range(HPC):
                    pi, hp = h // 2, h % 2
                    hs = slice(hp * DK, (hp + 1) * DK)
                    oT = psum_pool.tile([P, 512], F32, tag="ps", name=f"oT{qc}{h}")
                    # software-pipelined: AV(i-1) is emitted after exp(i) so PE
                    # computes scores(i) while ACT runs exp(i-1)
                    av_prev = None
                    for i in range(n_k):
                        m = i - 4 * qc
                        c0 = P * m if m > 0 else 0  # diag tiles: cols < 128m fully masked
                        sc = psum_pool.tile([P, 512], F32, tag="ps", name=f"sc{qc}{h}{i}")
                        nc.tensor.matmul(
                            sc[:, c0:],
                            kt_sb[pi][hs, i * P:(i + 1) * P],
                            qt_sb[pi][hs, qlo + c0:qlo + 512],
                            start=True,
                            stop=True,
                        )
                        if m >= 0:  # diagonal tile: additive causal mask
                            nc.vector.tensor_add(
                                sc[:, c0:], sc[:, c0:], mask_sb[:, m * 512 + c0:(m + 1) * 512]
                            )
                        pr = probs_pool.tile([P, 512], BF16, tag="pr", name=f"pr{qc}{h}{i}")
                        nc.scalar.activation(
                            pr[:, c0:], sc[:, c0:], mybir.ActivationFunctionType.Exp, scale=SCALE
                        )
                        if av_prev is not None:
                            j, d0, prj = av_prev
                            nc.tensor.matmul(
                                oT[0:DK + 1, d0:],
                                vones[h][:, j * (DK + 1):(j + 1) * (DK + 1)],
                                prj[:, d0:],
                                start=(j == 0),
                                stop=False,
                            )
                        av_prev = (i, c0, pr)
                    j, d0, prj = av_prev
                    nc.tensor.matmul(
                        oT[0:DK + 1, d0:],
                        vones[h][:, j * (DK + 1):(j + 1) * (DK + 1)],
                        prj[:, d0:],
                        start=(j == 0),
                        stop=True,
                    )
                    # normalize: rows 0..63 are V.T@P.T, row 64 is the row-sums
                    ssum = small.tile([1, 512], F32, tag="ssum", name=f"ssum{qc}{h}")
                    nc.vector.tensor_copy(ssum[:], oT[DK:DK + 1, :])
                    bc = psum_pool.tile([P, 512], F32, tag="ps", name=f"bc{qc}{h}")
                    nc.tensor.matmul(bc[0:DK, :], ones64[:], ssum[:], start=True, stop=True)
                    rec = small.tile([DK, 512], F32, tag="rec", name=f"rec{qc}{h}")
                    nc.vector.reciprocal(rec[:], bc[0:DK, :])
                    nc.vector.tensor_mul(
                        otn_sb[pi][hp * DK:(hp + 1) * DK, qlo:qlo + 512],
                        oT[0:DK, :],
                        rec[:],
                    )
                # output projection for the 4 q-tiles of this chunk
                for t in range(4):
                    qs = qlo + t * P
                    for dchunk in range(2):
                        yp = psum_pool.tile([P, 512], F32, tag="ps", name=f"yp{qc}{t}{dchunk}")
                        for pi in range(NPAIR):
                            nc.tensor.matmul(
                                yp[:],
                                otn_sb[pi][:, qs:qs + P],
                                woT_sb[pi][:, dchunk * 512:(dchunk + 1) * 512],
                                start=(pi == 0),
                                stop=(pi == NPAIR - 1),
                            )
                        ys = yout.tile([P, 512], F32, tag="ys", name=f"ys{qc}{t}{dchunk}")
                        nc.vector.tensor_copy(ys[:], yp[:])
                        nc.sync.dma_start(y[qs:qs + P, dchunk * 512:(dchunk + 1) * 512], ys[:])

    nc.compile()
    return nc


def _mask_phases():
    c = np.arange(512)[None, :]
    p = np.arange(P)[:, None]
    return np.stack(
        [np.where(c >= p + P * m, 0.0, NEG) for m in range(4)]
    ).astype(np.float32)


def _prep_in_maps(query, key, value, Wq, Wk, Wv, Wo):
    xT = {}
    for b in range(B):
        xT[b] = tuple(
            np.ascontiguousarray(np.asarray(a[b], np.float32).T).astype(BF)
            for a in (query, key, value)
        )
    maskT = _mask_phases()
    in_maps = []
    for core in range(8):
        b, g = core // 4, core % 4
        hsl = slice(HPC * g, HPC * (g + 1))
        in_maps.append({
            "xqT": xT[b][0],
            "xkT": xT[b][1],
            "xvT": xT[b][2],
            "wq": np.concatenate(list(np.asarray(Wq, np.float32)[hsl]), axis=1).astype(BF),
            "wk": np.concatenate(list(np.asarray(Wk, np.float32)[hsl]), axis=1).astype(BF),
            "wv": np.concatenate(list(np.asarray(Wv, np.float32)[hsl]), axis=1).astype(BF),
            "woT": np.ascontiguousarray(
                np.asarray(Wo, np.float32)[:, 256 * g:256 * (g + 1)].T
            ).astype(BF),
            "maskT": maskT,
        })
    return in_maps


def run(query, key, value, Wq, Wk, Wv, Wo, bo, trace=False, **trace_kwargs):
    if "nc" not in _CACHE:
        _CACHE["nc"] = _build_program()
    nc = _CACHE["nc"]
    in_maps = _prep_in_maps(query, key, value, Wq, Wk, Wv, Wo)
    res = run_bass_kernel_spmd(nc, in_maps, list(range(8)), trace=trace, **trace_kwargs)
    out = np.zeros((B, S, D), np.float32)
    for core in range(8):
        out[core // 4] += np.asarray(res.results[core]["y"], np.float32)
    out += np.asarray(bo, np.float32)[None, None, :]
    return out, res


def kernel(query, key, value, Wq, Wk, Wv, Wo, bo):
    out, _ = run(query, key, value, Wq, Wk, Wv, Wo, bo)
    return out
